# revision 1
# baseline (speedup 1.0000x reference)
"""Trainium2 Bass kernel for the BinaryMechanismSSM problem.

Full inputs in, full outputs out. Internally: batch (128) sharded 8 ways
(16 rows/core). Per core:
  Phase 1: projections bx0/bx1/gx = x @ {B0,B1,G}^T + bias (f32r matmuls,
           N=512 token tiles), sigmoid applied to the gate, staged to DRAM
           planes proj[mat][j] = [128, T*16] (token = t*16 + b).
  Phase 2: T sequential steps. State held as st[p, 16j+b] = s[b, 128j+p]
           ([128, 64] tile). Per step: 32 fp16 matmuls (weight-stationary
           A-blocks, rhs = fp16 state slices) accumulate f0/f1 into
           [128, 64] PSUM; DVE adds the staged projections; ACT tanh;
           DVE blend + gate; per-step DMA of the new state to a staging
           buffer [T, 128, 64]. Host re-layouts to [B, T+1, S].
"""
import numpy as np

B_FULL = 128
T_FULL = 1024
I_DIM = 256
S_DIM = 512
N_CORES = 8
B_LOC = B_FULL // N_CORES  # 16

_cache = {}


def _build(alpha: float, z: int, T: int):
    import concourse.bass as bass
    from concourse import bacc
    import concourse.mybir as mybir
    from concourse.tile import TileContext

    dt = mybir.dt
    AF = mybir.ActivationFunctionType
    ALU = mybir.AluOpType

    TOK = T * B_LOC          # tokens per core
    NTT = TOK // 512         # phase-1 token tiles
    NG = T // 16             # phase-2 step groups
    NMAT = 3 if z != 0 else 2          # number of projection matrices
    NREC = 2 if z != 0 else 1          # number of recurrence matrices

    nc = bacc.Bacc("TRN2", target_bir_lowering=False, debug=False,
                   num_devices=N_CORES)

    xT_d = nc.declare_dram_parameter("xT", [2, 128, TOK], dt.float32r, isOutput=False)
    pw_d = nc.declare_dram_parameter("pw", [128, NMAT * 2 * 4 * 128], dt.float32r, isOutput=False)
    bias_d = nc.declare_dram_parameter("bias", [128, 4 * NMAT], dt.float32, isOutput=False)
    aw_d = nc.declare_dram_parameter("aw", [128, NREC * 16 * 128], dt.float16, isOutput=False)
    s0_d = nc.declare_dram_parameter("s0T", [128, 64], dt.float32, isOutput=False)
    iden_d = nc.declare_dram_parameter("iden", [128, 128], dt.float16, isOutput=False)
    stg_d = nc.declare_dram_parameter("stg", [T, 128, 64], dt.float32, isOutput=True)

    with TileContext(nc) as tc:
      with tc.tile_pool(name="dram", bufs=1, space="DRAM") as dpool:
        projh_p = [[dpool.tile([128, TOK], dt.float16, tag=f"projh{m}{j}",
                               name=f"projh{m}{j}")
                    for j in range(4)] for m in range(NREC)]
        projl_p = [[dpool.tile([128, TOK], dt.float16, tag=f"projl{m}{j}",
                               name=f"projl{m}{j}")
                    for j in range(4)] for m in range(NREC)]
        projg_p = [dpool.tile([128, TOK], dt.float32, tag=f"projg{j}",
                              name=f"projg{j}") for j in range(4)]
        # ---------------- Phase 1: projections ----------------
        with (
            tc.tile_pool(name="p1w", bufs=1) as p1w,
            tc.tile_pool(name="p1x", bufs=3) as p1x,
            tc.tile_pool(name="p1o", bufs=6) as p1o,
            tc.tile_pool(name="p1ps", bufs=8, space="PSUM") as p1ps,
        ):
            pw = p1w.tile([128, NMAT * 2 * 4 * 128], dt.float32r)
            nc.sync.dma_start(pw[:], pw_d[:])
            bias = p1w.tile([128, 4 * NMAT], dt.float32)
            nc.sync.dma_start(bias[:], bias_d[:])

            for tt in range(NTT):
                xt = p1x.tile([128, 2 * 512], dt.float32r, tag="xt")
                for i in range(2):
                    nc.sync.dma_start(xt[:, i * 512:(i + 1) * 512],
                                      xT_d[i, :, tt * 512:(tt + 1) * 512])
                for mat in range(NMAT):
                    for j in range(4):
                        ps = p1ps.tile([128, 512], dt.float32, tag="pps")
                        for i in range(2):
                            blk = ((mat * 2 + i) * 4 + j) * 128
                            nc.tensor.matmul(
                                ps[:], pw[:, blk:blk + 128],
                                xt[:, i * 512:(i + 1) * 512],
                                start=(i == 0), stop=(i == 1))
                        bj = bias[:, mat * 4 + j:mat * 4 + j + 1]
                        if mat == NMAT - 1:
                            ot = p1o.tile([128, 512], dt.float32, tag="po")
                            nc.scalar.activation(ot[:], ps[:], AF.Sigmoid,
                                                 bias=bj, scale=1.0)
                            nc.sync.dma_start(
                                projg_p[j][:, tt * 512:(tt + 1) * 512], ot[:])
                        else:
                            hi = p1o.tile([128, 512], dt.float16, tag="phi")
                            nc.scalar.activation(hi[:], ps[:], AF.Identity,
                                                 bias=bj, scale=1.0)
                            lo = p1o.tile([128, 512], dt.float16, tag="plo")
                            nc.vector.scalar_tensor_tensor(
                                lo[:], ps[:], bj, hi[:], ALU.add, ALU.subtract)
                            nc.sync.dma_start(
                                projh_p[mat][j][:, tt * 512:(tt + 1) * 512], hi[:])
                            nc.sync.dma_start(
                                projl_p[mat][j][:, tt * 512:(tt + 1) * 512], lo[:])

        # ---------------- Phase 2: recurrence ----------------
        with (
            tc.tile_pool(name="p2w", bufs=1) as p2w,
            tc.tile_pool(name="p2in", bufs=2) as p2in,
            tc.tile_pool(name="p2st", bufs=2) as p2st,
            tc.tile_pool(name="p2c", bufs=3) as p2c,
            tc.tile_pool(name="p2ps", bufs=4, space="PSUM") as p2ps,
        ):
            aw = p2w.tile([128, NREC * 16 * 128], dt.float16)
            nc.sync.dma_start(aw[:], aw_d[:])
            iden = p2w.tile([128, 128], dt.float16)
            nc.sync.dma_start(iden[:], iden_d[:])

            st = p2st.tile([128, 64], dt.float32, tag="st")
            nc.sync.dma_start(st[:], s0_d[:])
            st16 = p2st.tile([128, 64], dt.float16, tag="st16")
            nc.scalar.activation(st16[:], st[:], AF.Copy)

            GATE_MAT = NMAT - 1
            a0 = float(1.0 - alpha) if z != 0 else 1.0
            a1 = float(alpha)

            for g in range(NG):
                # staged bx planes (f32r for the identity-MM injection)
                # contiguous hi/lo fp16 staging: (h, m, j, t, b)
                pjb = p2in.tile([128, 2 * NREC * 4 * 256], dt.float16, tag="pjb")
                for h, planes in enumerate((projh_p, projl_p)):
                    for m in range(NREC):
                        for j in range(4):
                            nc.sync.dma_start(
                                pjb[:, ((h * NREC + m) * 4 + j) * 256:
                                       ((h * NREC + m) * 4 + j + 1) * 256],
                                planes[m][j][:, g * 256:(g + 1) * 256])
                pjbr = pjb[:].rearrange("p (h m j t b) -> p h m j t b",
                                        h=2, m=NREC, j=4, t=16, b=16)
                # staged gate plane (fp32 for DVE)
                pjg = p2in.tile([128, 4 * 256], dt.float32, tag="pjg")
                for j in range(4):
                    nc.sync.dma_start(
                        pjg[:, j * 256:(j + 1) * 256],
                        projg_p[j][:, g * 256:(g + 1) * 256])

                # per-group gate coefficient planes (off the serial path):
                # gco[:, m-block] = coef_m * g ; g1m = 1 - g
                gco = p2in.tile([128, NREC * 1024], dt.float32, tag="gco")
                nc.vector.tensor_scalar_mul(gco[:, 0:1024], pjg[:], a0)
                if NREC == 2:
                    nc.vector.tensor_scalar_mul(gco[:, 1024:2048], pjg[:], a1)
                gcor = gco[:].rearrange("p (m j t b) -> p m j t b",
                                        m=NREC, j=4, t=16, b=16)
                g1m = p2in.tile([128, 1024], dt.float32, tag="g1m")
                nc.vector.tensor_scalar(g1m[:], pjg[:], -1.0, 1.0,
                                        ALU.mult, ALU.add)
                g1mr = g1m[:].rearrange("p (j t b) -> p j t b", j=4, t=16)

                for tt in range(16):
                    t = g * 16 + tt
                    W = NREC * 64
                    pscat = p2ps.tile([128, W], dt.float32, tag="pscat")
                    # inject bx = hi + lo via fp16 identity matmuls
                    for m in range(NREC):
                        for h in range(2):
                            nc.tensor.matmul(
                                pscat[:, m * 64:(m + 1) * 64]
                                .rearrange("p (j b) -> p j b", j=4),
                                iden[:], pjbr[:, h, m, :, tt, :],
                                start=(m == 0 and h == 0), stop=False)
                    # m2 = (1-g) * s  (off serial path, only needs st)
                    m2 = p2c.tile([128, 64], dt.float32, tag="m2")
                    nc.vector.tensor_tensor(
                        m2[:].rearrange("p (j b) -> p j b", j=4),
                        st[:].rearrange("p (j b) -> p j b", j=4),
                        g1mr[:, :, tt, :], ALU.mult)
                    # A matmuls accumulate on top
                    for m in range(NREC):
                        for j in range(4):
                            for k in range(4):
                                blk = (m * 16 + k * 4 + j) * 128
                                nc.tensor.matmul(
                                    pscat[:, (m * 4 + j) * 16:(m * 4 + j + 1) * 16],
                                    aw[:, blk:blk + 128],
                                    st16[:, k * 16:(k + 1) * 16],
                                    start=False,
                                    stop=(k == 3))
                    # one tanh over the whole [128, NREC*64] psum
                    ft = p2c.tile([128, W], dt.float32, tag="ft")
                    nc.scalar.activation(ft[:], pscat[:], AF.Tanh)
                    # mcat = gco_t * ft
                    mc = p2c.tile([128, W], dt.float32, tag="mc")
                    nc.vector.tensor_tensor(
                        mc[:].rearrange("p (m j b) -> p m j b", m=NREC, j=4),
                        ft[:].rearrange("p (m j b) -> p m j b", m=NREC, j=4),
                        gcor[:, :, :, tt, :], ALU.mult)
                    # reduce + new state (fp16 copy gates next step's matmuls)
                    if NREC == 2:
                        t2 = p2c.tile([128, 64], dt.float32, tag="t2")
                        nc.vector.tensor_tensor(t2[:], mc[:, 0:64], mc[:, 64:128],
                                                ALU.add)
                    else:
                        t2 = mc
                    st16_new = p2st.tile([128, 64], dt.float16, tag="st16")
                    nc.vector.tensor_tensor(st16_new[:], t2[:], m2[:], ALU.add)
                    st_new = p2st.tile([128, 64], dt.float32, tag="st")
                    nc.vector.tensor_tensor(st_new[:], t2[:], m2[:], ALU.add)
                    st, st16 = st_new, st16_new

                    nc.sync.dma_start(stg_d[t], st[:])

    nc.compile()
    return nc


def _pack_lhsT_blocks(W, kdim, mdim, dtype):
    """W: [mdim*128, kdim*128]; returns [128, kdim*mdim*128] with block
    (k, j) at cols (k*mdim+j)*128 equal to W[j-chunk, k-chunk].T."""
    nk, nj = kdim, mdim
    out = np.zeros((128, nk * nj * 128), dtype=dtype)
    for k in range(nk):
        for j in range(nj):
            blk = W[j * 128:(j + 1) * 128, k * 128:(k + 1) * 128].T
            out[:, (k * nj + j) * 128:(k * nj + j + 1) * 128] = blk
    return np.ascontiguousarray(out)


def kernel(x_seq, s0, A0_w, B0_w, B0_b, A1_w, B1_w, B1_b, gate_w, gate_b,
           alpha, z, _T=None, _trace=False):
    from concourse.bass_utils import run_bass_kernel_spmd

    T = int(_T or T_FULL)
    alpha_f = float(np.asarray(alpha))
    z_i = int(np.asarray(z))

    key = (alpha_f, z_i, T)
    if key not in _cache:
        _cache[key] = _build(alpha_f, z_i, T)
    nc = _cache[key]

    NMAT = 3 if z_i != 0 else 2
    NREC = 2 if z_i != 0 else 1

    x_seq = np.asarray(x_seq, dtype=np.float32)
    s0 = np.asarray(s0, dtype=np.float32)

    # ---- shared (replicated) weight packing ----
    # pw: phase-1 lhsT blocks per matrix: (mat, i, j) at col ((mat*2+i)*4+j)*128
    mats = [np.asarray(B0_w), np.asarray(B1_w), np.asarray(gate_w)][:NMAT] \
        if z_i != 0 else [np.asarray(B0_w), np.asarray(gate_w)]
    biases = [np.asarray(B0_b), np.asarray(B1_b), np.asarray(gate_b)][:NMAT] \
        if z_i != 0 else [np.asarray(B0_b), np.asarray(gate_b)]
    pw = np.concatenate(
        [_pack_lhsT_blocks(W.astype(np.float32), 2, 4, np.float32).reshape(128, 2, 4 * 128).reshape(128, -1)
         for W in mats], axis=1)
    # note: _pack_lhsT_blocks already gives (i*4+j) ordering per matrix
    pw = np.ascontiguousarray(pw)

    bias = np.zeros((128, 4 * NMAT), np.float32)
    for mi, bvec in enumerate(biases):
        bias[:, mi * 4:(mi + 1) * 4] = bvec.astype(np.float32).reshape(4, 128).T

    recs = [np.asarray(A0_w)] if z_i == 0 else [np.asarray(A0_w), np.asarray(A1_w)]
    aw = np.concatenate(
        [_pack_lhsT_blocks(A.astype(np.float32), 4, 4, np.float32)
         for A in recs], axis=1).astype(np.float16)
    aw = np.ascontiguousarray(aw)

    IDEN = np.ascontiguousarray(np.eye(128, dtype=np.float16))

    # ---- per-core inputs ----
    in_maps = []
    for c in range(N_CORES):
        bc = c * B_LOC
        xc = x_seq[bc:bc + B_LOC, :T]                       # [16, T, 256]
        xT = np.ascontiguousarray(
            xc.transpose(2, 1, 0).reshape(2, 128, T * B_LOC))
        s0c = s0[bc:bc + B_LOC]                             # [16, 512]
        s0T = np.ascontiguousarray(
            s0c.T.reshape(4, 128, B_LOC).transpose(1, 0, 2).reshape(128, 64))
        in_maps.append({
            "xT": xT, "pw": pw, "bias": bias, "aw": aw, "s0T": s0T,
            "iden": IDEN,
        })

    res = run_bass_kernel_spmd(nc, in_maps, list(range(N_CORES)), trace=_trace)
    if _trace:
        kernel._last_res = res

    out = np.empty((B_FULL, T + 1, S_DIM), np.float32)
    for c in range(N_CORES):
        bc = c * B_LOC
        stg = res.results[c]["stg"]                         # [T, 128, 64]
        out[bc:bc + B_LOC, 0] = s0[bc:bc + B_LOC]
        out[bc:bc + B_LOC, 1:] = (
            stg.reshape(T, 128, 4, B_LOC).transpose(3, 0, 2, 1)
            .reshape(B_LOC, T, S_DIM))
    return out



# revision 26
# speedup vs baseline: 1.1799x; 1.1799x over previous
"""Trainium2 Bass kernel for the BinaryMechanismSSM problem.

Full inputs in, full outputs out. Internally: batch (128) sharded 8 ways
(16 rows/core). Per core, a single fused pass:
  Projections (phase 1) are interleaved into the recurrence's idle PE/DVE/
  ACT windows: per 512-token tile, f32r matmuls compute bx0/bx1 (bias add
  + fp16 cast on DVE) and the gate planes gco = gcoef*sigmoid(x) (Pool)
  and g1m = sigmoid(-x) (ACT), packed into per-group-layout SBUF tiles and
  shipped to DRAM staging with one DMA each.
  Recurrence: T sequential steps, feature-block (j) pipelined. State lives
  as fp16 slices of a per-group staging tile stg_sb[p, t*64+j*16+b]
  (s[b, 128j+p] after step t). Per step: 4 psum tiles [128, 2*16] (one per
  feature block j); 1 fp16 identity matmul injects bx, 8 fp16 A-matmuls
  accumulate (each block consumes state block k in cyclic order ending
  with k=j); per-block tail tanh (ACT) -> mix/blend. Late-closing blocks
  j2/j3 run their blend on DVE, early blocks j0/j1 on GPSIMD; the (1-g)*s
  term is computed per half on the engine that produced that state half.
  One DMA per group ships 16 steps of states to DRAM; host re-layouts to
  [B, T+1, S].
"""
import numpy as np

B_FULL = 128
T_FULL = 1024
I_DIM = 256
S_DIM = 512
N_CORES = 8
B_LOC = B_FULL // N_CORES  # 16

_cache = {}


def _build(alpha: float, z: int, T: int):
    import concourse.bass as bass
    from concourse import bacc
    import concourse.mybir as mybir
    from concourse.tile import TileContext

    dt = mybir.dt
    AF = mybir.ActivationFunctionType
    ALU = mybir.AluOpType

    TOK = T * B_LOC          # tokens per core
    NTT = TOK // 512         # phase-1 token tiles (32 steps each)
    NG = T // 16             # phase-2 step groups
    NREC = 2 if z != 0 else 1
    NMAT = NREC + 1
    W = NREC * 16            # psum tile width per j block
    LAG = 2                  # tiles of projection lead

    # gate fold: st = gco * u + g1m * s, gco = gcoef * sigmoid, with
    #   alpha <= 0.5: gcoef = 1-alpha, u = ft0 + gam*ft1, gam = a/(1-a)
    #   alpha >  0.5: gcoef = alpha,   u = bet*ft0 + ft1, bet = (1-a)/a
    if NREC == 1:
        gcoef, mixc, mix_lo = 1.0, None, None
    elif alpha <= 0.5:
        gcoef, mixc, mix_lo = 1.0 - alpha, alpha / (1.0 - alpha), True
    else:
        gcoef, mixc, mix_lo = alpha, (1.0 - alpha) / alpha, False

    nc = bacc.Bacc("TRN2", target_bir_lowering=False, debug=False,
                   num_devices=N_CORES)

    xT_d = nc.declare_dram_parameter("xT", [2, 128, TOK], dt.float32r, isOutput=False)
    pw_d = nc.declare_dram_parameter("pw", [128, NMAT * 2 * 4 * 128], dt.float32r, isOutput=False)
    bias_d = nc.declare_dram_parameter("bias", [128, 4 * NMAT + 4], dt.float32, isOutput=False)
    aw_d = nc.declare_dram_parameter("aw", [128, NREC * 16 * 128], dt.float16, isOutput=False)
    s0_d = nc.declare_dram_parameter("s0T", [128, 64], dt.float16, isOutput=False)
    iden_d = nc.declare_dram_parameter("iden", [128, 128], dt.float16, isOutput=False)
    stg_d = nc.declare_dram_parameter("stg", [128, T * 64], dt.float16, isOutput=True)

    with TileContext(nc) as tc:
      with tc.tile_pool(name="dram", bufs=1, space="DRAM") as dpool:
        # bx staging: col = t*(NREC*64) + m*64 + j*16 + b
        pjb_d = dpool.tile([128, T * NREC * 64], dt.float16, tag="pjbd",
                           name="pjbd")
        # gate staging: col = t*128 + kind*64 + j*16 + b  (0=gco, 1=g1m)
        gat_d = dpool.tile([128, T * 128], dt.float16, tag="gatd", name="gatd")

        with (
            tc.tile_pool(name="wp", bufs=1) as wp,
            tc.tile_pool(name="p1x", bufs=3) as p1x,
            tc.tile_pool(name="p1o", bufs=3) as p1o,
            tc.tile_pool(name="p1ps", bufs=3, space="PSUM") as p1ps,
            tc.tile_pool(name="p2in", bufs=2) as p2in,
            tc.tile_pool(name="p2stg", bufs=2) as p2stg,
            tc.tile_pool(name="p2c", bufs=16) as p2c,
            tc.tile_pool(name="p2ps", bufs=1, space="PSUM") as p2ps,
        ):
            pw = wp.tile([128, NMAT * 2 * 4 * 128], dt.float32r)
            nc.sync.dma_start(pw[:], pw_d[:])
            bias = wp.tile([128, 4 * NMAT + 4], dt.float32)
            nc.sync.dma_start(bias[:], bias_d[:])
            gcoef_t = wp.tile([128, 256], dt.float16)
            nc.gpsimd.memset(gcoef_t[:], gcoef)
            aw = wp.tile([128, NREC * 16 * 128], dt.float16)
            nc.sync.dma_start(aw[:], aw_d[:])
            iden = wp.tile([128, 128], dt.float16)
            nc.sync.dma_start(iden[:], iden_d[:])
            s0sb = wp.tile([128, 64], dt.float16)
            nc.sync.dma_start(s0sb[:], s0_d[:])

            # ---------------- projection emitters ----------------
            p1st = {}

            def p1_prefetch(tt):
                if tt >= NTT:
                    return
                xt = p1x.tile([128, 2 * 512], dt.float32r, tag="xt")
                for i in range(2):
                    nc.sync.dma_start(xt[:, i * 512:(i + 1) * 512],
                                      xT_d[i, :, tt * 512:(tt + 1) * 512])
                p1st[tt] = {"xt": xt}

            def p1_unit(tt, u):
                """u in 0..NMAT*4-1 -> (mat, j): 2 matmuls + post ops."""
                if tt >= NTT:
                    return
                st = p1st[tt]
                if u == 0:
                    st["pjbpk"] = p1o.tile([128, 32 * NREC * 64], dt.float16,
                                           tag="pjbpk", name="pjbpk")
                    st["gatpk"] = p1o.tile([128, 32 * 128], dt.float16,
                                           tag="gatpk", name="gatpk")
                mat, j = divmod(u, 4)
                pjbpk_r = st["pjbpk"][:].rearrange(
                    "p (t m j b) -> p t m j b", t=32, m=NREC, j=4)
                gatpk_r = st["gatpk"][:].rearrange(
                    "p (t k j b) -> p t k j b", t=32, k=2, j=4)
                ps = p1ps.tile([128, 512], dt.float32, tag="pps")
                for i in range(2):
                    blk = ((mat * 2 + i) * 4 + j) * 128
                    nc.tensor.matmul(ps[:], pw[:, blk:blk + 128],
                                     st["xt"][:, i * 512:(i + 1) * 512],
                                     start=(i == 0), stop=(i == 1))
                psr = ps[:].rearrange("p (t b) -> p t b", t=32)
                bj = bias[:, mat * 4 + j:mat * 4 + j + 1]
                if mat == NMAT - 1:
                    g16 = p1o.tile([128, 512], dt.float16, tag="g16")
                    g16r = g16[:].rearrange("p (t b) -> p t b", t=32)
                    nbj = bias[:, 4 * NMAT + j:4 * NMAT + j + 1]
                    for h in range(2):
                        hs = slice(h * 16, (h + 1) * 16)
                        nc.scalar.activation(g16r[:, hs], psr[:, hs],
                                             AF.Sigmoid, bias=bj, scale=1.0)
                        nc.scalar.activation(gatpk_r[:, hs, 1, j, :],
                                             psr[:, hs], AF.Sigmoid,
                                             bias=nbj, scale=-1.0)
                        nc.gpsimd.tensor_tensor(
                            gatpk_r[:, hs, 0, j, :], g16r[:, hs],
                            gcoef_t[:].rearrange("p (t b) -> p t b", t=16),
                            ALU.mult)
                else:
                    for h in range(2):
                        hs = slice(h * 16, (h + 1) * 16)
                        nc.vector.tensor_scalar(pjbpk_r[:, hs, mat, j, :],
                                                psr[:, hs], bj, None, ALU.add)

            def p1_flush(tt):
                if tt >= NTT:
                    return
                st = p1st.pop(tt)
                GW32 = 32 * NREC * 64
                nc.sync.dma_start(pjb_d[:, tt * GW32:(tt + 1) * GW32],
                                  st["pjbpk"][:])
                nc.sync.dma_start(gat_d[:, tt * 4096:(tt + 1) * 4096],
                                  st["gatpk"][:])

            # prologue: first LAG tiles of projections
            NU = NMAT * 4
            for tt in range(min(LAG, NTT)):
                p1_prefetch(tt)
                for u in range(NU):
                    p1_unit(tt, u)
                p1_flush(tt)
            p1_prefetch(LAG)

            # ---------------- recurrence ----------------
            GW = 16 * NREC * 64
            prev_stg = None
            for g in range(NG):
                pjb = p2in.tile([128, GW], dt.float16, tag="pjb")
                nc.sync.dma_start(pjb[:], pjb_d[:, g * GW:(g + 1) * GW])
                pjb_r = pjb[:].rearrange("p (t m j b) -> p m j t b",
                                         t=16, m=NREC, j=4)
                gat = p2in.tile([128, 2048], dt.float16, tag="gat")
                nc.sync.dma_start(gat[:], gat_d[:, g * 2048:(g + 1) * 2048])

                stg = p2stg.tile([128, 16 * 64], dt.float16, tag="stg")

                # projection work interleaved into this group
                tile_idx = g // 2 + LAG
                ubase = 0 if g % 2 == 0 else NU - NU // 2
                units = list(range(ubase, min(ubase + NU - NU // 2, NU)))

                for tt in range(16):
                    if g == 0 and tt == 0:
                        st_tile, st_off = s0sb, 0
                    elif tt == 0:
                        st_tile, st_off = prev_stg, 15 * 64
                    else:
                        st_tile, st_off = stg, (tt - 1) * 64

                    # m2 halves: each engine blends the half it produced
                    m2p = p2c.tile([128, 32], dt.float16, tag="m2p")
                    nc.gpsimd.tensor_tensor(
                        m2p[:], st_tile[:, st_off:st_off + 32],
                        gat[:, tt * 128 + 64:tt * 128 + 96], ALU.mult)
                    m2v = p2c.tile([128, 32], dt.float16, tag="m2v")
                    nc.vector.tensor_tensor(
                        m2v[:], st_tile[:, st_off + 32:st_off + 64],
                        gat[:, tt * 128 + 96:tt * 128 + 128], ALU.mult)

                    fts = []
                    for j in range(4):
                        psj = p2ps.tile([128, W], dt.float32, tag=f"ps{j}")
                        nc.tensor.matmul(
                            psj[:], iden[:], pjb_r[:, :, j, tt, :],
                            start=True, stop=False)
                        for m in range(NREC):
                            for ki in range(4):
                                k = (j + 1 + ki) % 4
                                blk = (m * 16 + k * 4 + j) * 128
                                nc.tensor.matmul(
                                    psj[:, m * 16:(m + 1) * 16],
                                    aw[:, blk:blk + 128],
                                    st_tile[:, st_off + k * 16:
                                            st_off + (k + 1) * 16],
                                    start=False, stop=(ki == 3))
                        ft = p2c.tile([128, W], dt.float16, tag=f"ft{j}")
                        nc.scalar.activation(ft[:], psj[:], AF.Tanh)
                        fts.append(ft)
                        if j == 1:
                            # unblock the GPSIMD chains for j0/j1 early
                            us = [_emit_mix(nc, mybir, p2c, fts[jj], jj,
                                            NREC, mixc, mix_lo)
                                  for jj in range(2)]
                            for jj in range(2):
                                _emit_blend(nc.gpsimd, mybir, p2c, us[jj],
                                            jj, gat, m2p, stg, tt)
                        elif j >= 2:
                            u = _emit_mix(nc, mybir, p2c, ft, j, NREC,
                                          mixc, mix_lo)
                            _emit_blend(nc.vector, mybir, p2c, u, j, gat,
                                        m2v, stg, tt)

                    # projection work in the PE stall window at step end
                    if tt % 2 == 1 and tt // 2 < len(units):
                        p1_unit(tile_idx, units[tt // 2])
                    if tt == 14 and g % 2 == 1:
                        p1_flush(tile_idx)
                        p1_prefetch(tile_idx + 1)

                nc.sync.dma_start(stg_d[:, g * 1024:(g + 1) * 1024], stg[:])
                prev_stg = stg

    nc.compile()
    return nc


def _emit_mix(nc, mybir, pool, ft, j, NREC, mixc, mix_lo):
    """u_j = mix of the two tanh halves (DVE STT); returns the u tile."""
    dt = mybir.dt
    ALU = mybir.AluOpType
    if NREC == 1:
        return ft
    u = pool.tile([128, 16], dt.float16, tag=f"u{j}", name=f"u{j}")
    if mix_lo:
        nc.vector.scalar_tensor_tensor(u[:], ft[:, 16:32], mixc, ft[:, 0:16],
                                       ALU.mult, ALU.add)
    else:
        nc.vector.scalar_tensor_tensor(u[:], ft[:, 0:16], mixc, ft[:, 16:32],
                                       ALU.mult, ALU.add)
    return u


def _emit_blend(eng, mybir, pool, u, j, gat, m2h, stg, tt):
    """mc_j = u_j * gco_j ; st_j = mc_j + m2_j -> state slice (TTs)."""
    dt = mybir.dt
    ALU = mybir.AluOpType
    mc = pool.tile([128, 16], dt.float16, tag=f"mc{j}", name=f"mc{j}")
    eng.tensor_tensor(mc[:], u[:, 0:16],
                      gat[:, tt * 128 + j * 16:tt * 128 + (j + 1) * 16],
                      ALU.mult)
    eng.tensor_tensor(stg[:, tt * 64 + j * 16:tt * 64 + (j + 1) * 16],
                      mc[:], m2h[:, (j % 2) * 16:(j % 2) * 16 + 16], ALU.add)


def _pack_lhsT_blocks(W, kdim, mdim, dtype):
    """W: [mdim*128, kdim*128]; returns [128, kdim*mdim*128] with block
    (k, j) at cols (k*mdim+j)*128 equal to W[j-chunk, k-chunk].T."""
    nk, nj = kdim, mdim
    out = np.zeros((128, nk * nj * 128), dtype=dtype)
    for k in range(nk):
        for j in range(nj):
            blk = W[j * 128:(j + 1) * 128, k * 128:(k + 1) * 128].T
            out[:, (k * nj + j) * 128:(k * nj + j + 1) * 128] = blk
    return np.ascontiguousarray(out)


def kernel(x_seq, s0, A0_w, B0_w, B0_b, A1_w, B1_w, B1_b, gate_w, gate_b,
           alpha, z, _T=None, _trace=False):
    from concourse.bass_utils import run_bass_kernel_spmd

    T = int(_T or T_FULL)
    alpha_f = float(np.asarray(alpha))
    z_i = int(np.asarray(z))

    key = (alpha_f, z_i, T)
    if key not in _cache:
        _cache[key] = _build(alpha_f, z_i, T)
    nc = _cache[key]

    NREC = 2 if z_i != 0 else 1
    NMAT = NREC + 1

    x_seq = np.asarray(x_seq, dtype=np.float32)
    s0 = np.asarray(s0, dtype=np.float32)

    # ---- shared (replicated) weight packing ----
    mats = [np.asarray(B0_w), np.asarray(B1_w), np.asarray(gate_w)] \
        if z_i != 0 else [np.asarray(B0_w), np.asarray(gate_w)]
    biases = [np.asarray(B0_b), np.asarray(B1_b), np.asarray(gate_b)] \
        if z_i != 0 else [np.asarray(B0_b), np.asarray(gate_b)]
    pw = np.concatenate(
        [_pack_lhsT_blocks(W.astype(np.float32), 2, 4, np.float32)
         for W in mats], axis=1)
    pw = np.ascontiguousarray(pw)

    bias = np.zeros((128, 4 * NMAT + 4), np.float32)
    for mi, bvec in enumerate(biases):
        bias[:, mi * 4:(mi + 1) * 4] = bvec.astype(np.float32).reshape(4, 128).T
    # negated gate bias (for g1m = sigmoid(-x) on ACT with scale=-1)
    bias[:, 4 * NMAT:] = -bias[:, (NMAT - 1) * 4:NMAT * 4]

    recs = [np.asarray(A0_w)] if z_i == 0 else [np.asarray(A0_w), np.asarray(A1_w)]
    aw = np.concatenate(
        [_pack_lhsT_blocks(A.astype(np.float32), 4, 4, np.float32)
         for A in recs], axis=1).astype(np.float16)
    aw = np.ascontiguousarray(aw)

    IDEN = np.ascontiguousarray(np.eye(128, dtype=np.float16))

    # ---- per-core inputs ----
    in_maps = []
    for c in range(N_CORES):
        bc = c * B_LOC
        xc = x_seq[bc:bc + B_LOC, :T]                       # [16, T, 256]
        xT = np.ascontiguousarray(
            xc.transpose(2, 1, 0).reshape(2, 128, T * B_LOC))
        s0c = s0[bc:bc + B_LOC]                             # [16, 512]
        s0T = np.ascontiguousarray(
            s0c.T.reshape(4, 128, B_LOC).transpose(1, 0, 2).reshape(128, 64)
        ).astype(np.float16)
        in_maps.append({
            "xT": xT, "pw": pw, "bias": bias, "aw": aw, "s0T": s0T,
            "iden": IDEN,
        })

    res = run_bass_kernel_spmd(nc, in_maps, list(range(N_CORES)), trace=_trace)
    if _trace:
        kernel._last_res = res

    out = np.empty((B_FULL, T + 1, S_DIM), np.float32)
    for c in range(N_CORES):
        bc = c * B_LOC
        stg = res.results[c]["stg"]                         # [128, T*64] fp16
        out[bc:bc + B_LOC, 0] = s0[bc:bc + B_LOC]
        out[bc:bc + B_LOC, 1:] = (
            stg.reshape(128, T, 4, B_LOC).transpose(3, 1, 2, 0)
            .reshape(B_LOC, T, S_DIM).astype(np.float32))
    return out


# revision 29
# speedup vs baseline: 1.2778x; 1.0829x over previous
"""Trainium2 Bass kernel for the BinaryMechanismSSM problem.

Full inputs in, full outputs out. Internally: batch (128) sharded 8 ways
(16 rows/core). Per core, a single fused pass:
  Projections (phase 1) are interleaved into the recurrence's idle PE/DVE/
  ACT windows: per 512-token tile, f32r matmuls compute bx0/bx1 (bias add
  + fp16 cast on DVE) and the gate planes gco = gcoef*sigmoid(x) (Pool)
  and g1m = sigmoid(-x) (ACT), packed into per-group-layout SBUF tiles and
  shipped to DRAM staging with one DMA each.
  Recurrence: T sequential steps, feature-block (j) pipelined. State lives
  as fp16 slices of a per-group staging tile stg_sb[p, t*64+j*16+b]
  (s[b, 128j+p] after step t). Per step: 4 psum tiles [128, 2*16] (one per
  feature block j); 1 fp16 identity matmul injects bx, 8 fp16 A-matmuls
  accumulate (each block consumes state block k in cyclic order ending
  with k=j); per-block tail tanh (ACT) -> mix/blend. Late-closing blocks
  j2/j3 run their blend on DVE, early blocks j0/j1 on GPSIMD; the (1-g)*s
  term is computed per half on the engine that produced that state half.
  One DMA per group ships 16 steps of states to DRAM; host re-layouts to
  [B, T+1, S].
"""
import numpy as np

B_FULL = 128
T_FULL = 1024
I_DIM = 256
S_DIM = 512
N_CORES = 8
B_LOC = B_FULL // N_CORES  # 16

_cache = {}


def _build(alpha: float, z: int, T: int):
    import concourse.bass as bass
    from concourse import bacc
    import concourse.mybir as mybir
    from concourse.tile import TileContext

    dt = mybir.dt
    AF = mybir.ActivationFunctionType
    ALU = mybir.AluOpType

    TOK = T * B_LOC          # tokens per core
    NTT = TOK // 512         # phase-1 token tiles (32 steps each)
    NG = T // 16             # phase-2 step groups
    NREC = 2 if z != 0 else 1
    NMAT = NREC + 1
    W = NREC * 16            # psum tile width per j block
    LAG = 2                  # tiles of projection lead

    # gate fold: st = gco * u + g1m * s, gco = gcoef * sigmoid, with
    #   alpha <= 0.5: gcoef = 1-alpha, u = ft0 + gam*ft1, gam = a/(1-a)
    #   alpha >  0.5: gcoef = alpha,   u = bet*ft0 + ft1, bet = (1-a)/a
    if NREC == 1:
        gcoef, mixc, mix_lo = 1.0, None, None
    elif alpha <= 0.5:
        gcoef, mixc, mix_lo = 1.0 - alpha, alpha / (1.0 - alpha), True
    else:
        gcoef, mixc, mix_lo = alpha, (1.0 - alpha) / alpha, False

    nc = bacc.Bacc("TRN2", target_bir_lowering=False, debug=False,
                   num_devices=N_CORES)

    xT_d = nc.declare_dram_parameter("xT", [2, 128, TOK], dt.float32r, isOutput=False)
    pw_d = nc.declare_dram_parameter("pw", [128, NMAT * 2 * 4 * 128], dt.float32r, isOutput=False)
    bias_d = nc.declare_dram_parameter("bias", [128, 4 * NMAT + 4], dt.float32, isOutput=False)
    aw_d = nc.declare_dram_parameter("aw", [128, NREC * 16 * 128], dt.float16, isOutput=False)
    s0_d = nc.declare_dram_parameter("s0T", [128, 64], dt.float16, isOutput=False)
    iden_d = nc.declare_dram_parameter("iden", [128, 128], dt.float16, isOutput=False)
    stg_d = nc.declare_dram_parameter("stg", [128, T * 64], dt.float16, isOutput=True)

    with TileContext(nc) as tc:
      with tc.tile_pool(name="dram", bufs=1, space="DRAM") as dpool:
        # bx staging: col = t*(NREC*64) + m*64 + j*16 + b
        pjb_d = dpool.tile([128, T * NREC * 64], dt.float16, tag="pjbd",
                           name="pjbd")
        # gate staging: col = t*128 + kind*64 + j*16 + b  (0=gco, 1=g1m)
        gat_d = dpool.tile([128, T * 128], dt.float16, tag="gatd", name="gatd")

        with (
            tc.tile_pool(name="wp", bufs=1) as wp,
            tc.tile_pool(name="p1x", bufs=3) as p1x,
            tc.tile_pool(name="p1o", bufs=3) as p1o,
            tc.tile_pool(name="p1ps", bufs=3, space="PSUM") as p1ps,
            tc.tile_pool(name="p2in", bufs=2) as p2in,
            tc.tile_pool(name="p2stg", bufs=2) as p2stg,
            tc.tile_pool(name="p2c", bufs=16) as p2c,
            tc.tile_pool(name="p2ps", bufs=2, space="PSUM") as p2ps,
        ):
            pw = wp.tile([128, NMAT * 2 * 4 * 128], dt.float32r)
            nc.sync.dma_start(pw[:], pw_d[:])
            bias = wp.tile([128, 4 * NMAT + 4], dt.float32)
            nc.sync.dma_start(bias[:], bias_d[:])
            gcoef_t = wp.tile([128, 256], dt.float16)
            nc.gpsimd.memset(gcoef_t[:], gcoef)
            aw = wp.tile([128, NREC * 16 * 128], dt.float16)
            nc.sync.dma_start(aw[:], aw_d[:])
            iden = wp.tile([128, 128], dt.float16)
            nc.sync.dma_start(iden[:], iden_d[:])
            s0sb = wp.tile([128, 64], dt.float16)
            nc.sync.dma_start(s0sb[:], s0_d[:])

            # ---------------- projection emitters ----------------
            p1st = {}

            def p1_prefetch(tt):
                if tt >= NTT:
                    return
                xt = p1x.tile([128, 2 * 512], dt.float32r, tag="xt")
                for i in range(2):
                    nc.sync.dma_start(xt[:, i * 512:(i + 1) * 512],
                                      xT_d[i, :, tt * 512:(tt + 1) * 512])
                p1st[tt] = {"xt": xt}

            def p1_unit(tt, u):
                """u in 0..NMAT*4-1 -> (mat, j): 2 matmuls + post ops."""
                if tt >= NTT:
                    return
                st = p1st[tt]
                if u == 0:
                    st["pjbpk"] = p1o.tile([128, 32 * NREC * 64], dt.float16,
                                           tag="pjbpk", name="pjbpk")
                    st["gatpk"] = p1o.tile([128, 32 * 128], dt.float16,
                                           tag="gatpk", name="gatpk")
                mat, j = divmod(u, 4)
                pjbpk_r = st["pjbpk"][:].rearrange(
                    "p (t m j b) -> p t m j b", t=32, m=NREC, j=4)
                gatpk_r = st["gatpk"][:].rearrange(
                    "p (t k j b) -> p t k j b", t=32, k=2, j=4)
                ps = p1ps.tile([128, 512], dt.float32, tag="pps")
                for i in range(2):
                    blk = ((mat * 2 + i) * 4 + j) * 128
                    nc.tensor.matmul(ps[:], pw[:, blk:blk + 128],
                                     st["xt"][:, i * 512:(i + 1) * 512],
                                     start=(i == 0), stop=(i == 1))
                psr = ps[:].rearrange("p (t b) -> p t b", t=32)
                bj = bias[:, mat * 4 + j:mat * 4 + j + 1]
                if mat == NMAT - 1:
                    g16 = p1o.tile([128, 512], dt.float16, tag="g16")
                    g16r = g16[:].rearrange("p (t b) -> p t b", t=32)
                    nbj = bias[:, 4 * NMAT + j:4 * NMAT + j + 1]
                    for h in range(2):
                        hs = slice(h * 16, (h + 1) * 16)
                        nc.scalar.activation(g16r[:, hs], psr[:, hs],
                                             AF.Sigmoid, bias=bj, scale=1.0)
                        nc.scalar.activation(gatpk_r[:, hs, 1, j, :],
                                             psr[:, hs], AF.Sigmoid,
                                             bias=nbj, scale=-1.0)
                        nc.gpsimd.tensor_tensor(
                            gatpk_r[:, hs, 0, j, :], g16r[:, hs],
                            gcoef_t[:].rearrange("p (t b) -> p t b", t=16),
                            ALU.mult)
                else:
                    for h in range(2):
                        hs = slice(h * 16, (h + 1) * 16)
                        nc.vector.tensor_scalar(pjbpk_r[:, hs, mat, j, :],
                                                psr[:, hs], bj, None, ALU.add)

            def p1_flush(tt):
                if tt >= NTT:
                    return
                st = p1st.pop(tt)
                GW32 = 32 * NREC * 64
                nc.sync.dma_start(pjb_d[:, tt * GW32:(tt + 1) * GW32],
                                  st["pjbpk"][:])
                nc.sync.dma_start(gat_d[:, tt * 4096:(tt + 1) * 4096],
                                  st["gatpk"][:])

            # prologue: first LAG tiles of projections
            NU = NMAT * 4
            for tt in range(min(LAG, NTT)):
                p1_prefetch(tt)
                for u in range(NU):
                    p1_unit(tt, u)
                p1_flush(tt)
            p1_prefetch(LAG)

            # ---------------- recurrence ----------------
            GW = 16 * NREC * 64
            prev_stg = None
            for g in range(NG):
                pjb = p2in.tile([128, GW], dt.float16, tag="pjb")
                nc.sync.dma_start(pjb[:], pjb_d[:, g * GW:(g + 1) * GW])
                pjb_r = pjb[:].rearrange("p (t m j b) -> p m j t b",
                                         t=16, m=NREC, j=4)
                gat = p2in.tile([128, 2048], dt.float16, tag="gat")
                nc.sync.dma_start(gat[:], gat_d[:, g * 2048:(g + 1) * 2048])

                stg = p2stg.tile([128, 16 * 64], dt.float16, tag="stg")

                # projection work interleaved into this group
                tile_idx = g // 2 + LAG
                ubase = 0 if g % 2 == 0 else NU - NU // 2
                units = list(range(ubase, min(ubase + NU - NU // 2, NU)))

                for tt in range(16):
                    if g == 0 and tt == 0:
                        st_tile, st_off = s0sb, 0
                    elif tt == 0:
                        st_tile, st_off = prev_stg, 15 * 64
                    else:
                        st_tile, st_off = stg, (tt - 1) * 64

                    # m2 halves: each engine blends the half it produced
                    m2p = p2c.tile([128, 32], dt.float16, tag="m2p")
                    nc.gpsimd.tensor_tensor(
                        m2p[:], st_tile[:, st_off:st_off + 32],
                        gat[:, tt * 128 + 64:tt * 128 + 96], ALU.mult)
                    m2v = p2c.tile([128, 32], dt.float16, tag="m2v")
                    nc.vector.tensor_tensor(
                        m2v[:], st_tile[:, st_off + 32:st_off + 64],
                        gat[:, tt * 128 + 96:tt * 128 + 128], ALU.mult)

                    # two psum pairs; cols (m, jj, b); pair p = blocks 2p,2p+1
                    for p in range(2):
                        psp = p2ps.tile([128, 2 * W], dt.float32,
                                        tag=f"psp{p}", name=f"psp{p}")
                        nc.tensor.matmul(
                            psp[:].rearrange("q (m j b) -> q m j b",
                                             m=NREC, j=2),
                            iden[:], pjb_r[:, :, 2 * p:2 * p + 2, tt, :],
                            start=True, stop=False)
                        for jj in range(2):
                            j = 2 * p + jj
                            ks = [k for k in range(4) if k != j and k != 3]
                            if j != 3:
                                ks.append(3)
                            ks.append(j)
                            for m in range(NREC):
                                for ki, k in enumerate(ks):
                                    blk = (m * 16 + k * 4 + j) * 128
                                    nc.tensor.matmul(
                                        psp[:, (m * 2 + jj) * 16:
                                            (m * 2 + jj + 1) * 16],
                                        aw[:, blk:blk + 128],
                                        st_tile[:, st_off + k * 16:
                                                st_off + (k + 1) * 16],
                                        start=False, stop=(ki == 3))
                        ft = p2c.tile([128, 2 * W], dt.float16,
                                      tag=f"ftp{p}", name=f"ftp{p}")
                        nc.scalar.activation(ft[:], psp[:], AF.Tanh)
                        # mix on DVE; blend pair0 on GPSIMD, pair1 on DVE
                        if NREC == 2:
                            u = p2c.tile([128, 32], dt.float16,
                                         tag=f"up{p}", name=f"up{p}")
                            if mix_lo:
                                nc.vector.scalar_tensor_tensor(
                                    u[:], ft[:, 32:64], mixc, ft[:, 0:32],
                                    ALU.mult, ALU.add)
                            else:
                                nc.vector.scalar_tensor_tensor(
                                    u[:], ft[:, 0:32], mixc, ft[:, 32:64],
                                    ALU.mult, ALU.add)
                        else:
                            u = ft
                        eng = nc.gpsimd if p == 0 else nc.vector
                        m2h = m2p if p == 0 else m2v
                        mc = p2c.tile([128, 32], dt.float16,
                                      tag=f"mcp{p}", name=f"mcp{p}")
                        eng.tensor_tensor(
                            mc[:], u[:, 0:32],
                            gat[:, tt * 128 + p * 32:tt * 128 + p * 32 + 32],
                            ALU.mult)
                        eng.tensor_tensor(
                            stg[:, tt * 64 + p * 32:tt * 64 + p * 32 + 32],
                            mc[:], m2h[:], ALU.add)

                    # projection work in the PE stall window at step end
                    if tt % 2 == 1 and tt // 2 < len(units):
                        p1_unit(tile_idx, units[tt // 2])
                    if tt == 14 and g % 2 == 1:
                        p1_flush(tile_idx)
                        p1_prefetch(tile_idx + 1)

                nc.sync.dma_start(stg_d[:, g * 1024:(g + 1) * 1024], stg[:])
                prev_stg = stg

    nc.compile()
    return nc


def _pack_lhsT_blocks(W, kdim, mdim, dtype):
    """W: [mdim*128, kdim*128]; returns [128, kdim*mdim*128] with block
    (k, j) at cols (k*mdim+j)*128 equal to W[j-chunk, k-chunk].T."""
    nk, nj = kdim, mdim
    out = np.zeros((128, nk * nj * 128), dtype=dtype)
    for k in range(nk):
        for j in range(nj):
            blk = W[j * 128:(j + 1) * 128, k * 128:(k + 1) * 128].T
            out[:, (k * nj + j) * 128:(k * nj + j + 1) * 128] = blk
    return np.ascontiguousarray(out)


def kernel(x_seq, s0, A0_w, B0_w, B0_b, A1_w, B1_w, B1_b, gate_w, gate_b,
           alpha, z, _T=None, _trace=False):
    from concourse.bass_utils import run_bass_kernel_spmd

    T = int(_T or T_FULL)
    alpha_f = float(np.asarray(alpha))
    z_i = int(np.asarray(z))

    key = (alpha_f, z_i, T)
    if key not in _cache:
        _cache[key] = _build(alpha_f, z_i, T)
    nc = _cache[key]

    NREC = 2 if z_i != 0 else 1
    NMAT = NREC + 1

    x_seq = np.asarray(x_seq, dtype=np.float32)
    s0 = np.asarray(s0, dtype=np.float32)

    # ---- shared (replicated) weight packing ----
    mats = [np.asarray(B0_w), np.asarray(B1_w), np.asarray(gate_w)] \
        if z_i != 0 else [np.asarray(B0_w), np.asarray(gate_w)]
    biases = [np.asarray(B0_b), np.asarray(B1_b), np.asarray(gate_b)] \
        if z_i != 0 else [np.asarray(B0_b), np.asarray(gate_b)]
    pw = np.concatenate(
        [_pack_lhsT_blocks(W.astype(np.float32), 2, 4, np.float32)
         for W in mats], axis=1)
    pw = np.ascontiguousarray(pw)

    bias = np.zeros((128, 4 * NMAT + 4), np.float32)
    for mi, bvec in enumerate(biases):
        bias[:, mi * 4:(mi + 1) * 4] = bvec.astype(np.float32).reshape(4, 128).T
    # negated gate bias (for g1m = sigmoid(-x) on ACT with scale=-1)
    bias[:, 4 * NMAT:] = -bias[:, (NMAT - 1) * 4:NMAT * 4]

    recs = [np.asarray(A0_w)] if z_i == 0 else [np.asarray(A0_w), np.asarray(A1_w)]
    aw = np.concatenate(
        [_pack_lhsT_blocks(A.astype(np.float32), 4, 4, np.float32)
         for A in recs], axis=1).astype(np.float16)
    aw = np.ascontiguousarray(aw)

    IDEN = np.ascontiguousarray(np.eye(128, dtype=np.float16))

    # ---- per-core inputs ----
    in_maps = []
    for c in range(N_CORES):
        bc = c * B_LOC
        xc = x_seq[bc:bc + B_LOC, :T]                       # [16, T, 256]
        xT = np.ascontiguousarray(
            xc.transpose(2, 1, 0).reshape(2, 128, T * B_LOC))
        s0c = s0[bc:bc + B_LOC]                             # [16, 512]
        s0T = np.ascontiguousarray(
            s0c.T.reshape(4, 128, B_LOC).transpose(1, 0, 2).reshape(128, 64)
        ).astype(np.float16)
        in_maps.append({
            "xT": xT, "pw": pw, "bias": bias, "aw": aw, "s0T": s0T,
            "iden": IDEN,
        })

    res = run_bass_kernel_spmd(nc, in_maps, list(range(N_CORES)), trace=_trace)
    if _trace:
        kernel._last_res = res

    out = np.empty((B_FULL, T + 1, S_DIM), np.float32)
    for c in range(N_CORES):
        bc = c * B_LOC
        stg = res.results[c]["stg"]                         # [128, T*64] fp16
        out[bc:bc + B_LOC, 0] = s0[bc:bc + B_LOC]
        out[bc:bc + B_LOC, 1:] = (
            stg.reshape(128, T, 4, B_LOC).transpose(3, 1, 2, 0)
            .reshape(B_LOC, T, S_DIM).astype(np.float32))
    return out


# revision 31
# speedup vs baseline: 1.3494x; 1.0560x over previous
"""Trainium2 Bass kernel for the BinaryMechanismSSM problem.

Full inputs in, full outputs out. Internally: batch (128) sharded 8 ways
(16 rows/core). Per core, a single fused pass:
  Projections (phase 1) are interleaved into the recurrence's idle PE/DVE/
  ACT windows: per 512-token tile, f32r matmuls compute bx0/bx1 (bias add
  + fp16 cast on DVE) and the gate planes gco = gcoef*sigmoid(x) (Pool)
  and g1m = sigmoid(-x) (ACT), packed into per-group-layout SBUF tiles and
  shipped to DRAM staging with one DMA each.
  Recurrence: T sequential steps, feature-block (j) pipelined. State lives
  as fp16 slices of a per-group staging tile stg_sb[p, t*64+j*16+b]
  (s[b, 128j+p] after step t). Per step: 4 psum tiles [128, 2*16] (one per
  feature block j); 1 fp16 identity matmul injects bx, 8 fp16 A-matmuls
  accumulate (each block consumes state block k in cyclic order ending
  with k=j); per-block tail tanh (ACT) -> mix/blend. Late-closing blocks
  j2/j3 run their blend on DVE, early blocks j0/j1 on GPSIMD; the (1-g)*s
  term is computed per half on the engine that produced that state half.
  One DMA per group ships 16 steps of states to DRAM; host re-layouts to
  [B, T+1, S].
"""
import numpy as np

B_FULL = 128
T_FULL = 1024
I_DIM = 256
S_DIM = 512
N_CORES = 8
B_LOC = B_FULL // N_CORES  # 16

_cache = {}


def _build(alpha: float, z: int, T: int):
    import concourse.bass as bass
    from concourse import bacc
    import concourse.mybir as mybir
    from concourse.tile import TileContext

    dt = mybir.dt
    AF = mybir.ActivationFunctionType
    ALU = mybir.AluOpType

    TOK = T * B_LOC          # tokens per core
    NTT = TOK // 512         # phase-1 token tiles (32 steps each)
    NG = T // 16             # phase-2 step groups
    NREC = 2 if z != 0 else 1
    NMAT = NREC + 1
    W = NREC * 16            # psum tile width per j block
    LAG = 2                  # tiles of projection lead

    # gate fold: st = gco * u + g1m * s, gco = gcoef * sigmoid, with
    #   alpha <= 0.5: gcoef = 1-alpha, u = ft0 + gam*ft1, gam = a/(1-a)
    #   alpha >  0.5: gcoef = alpha,   u = bet*ft0 + ft1, bet = (1-a)/a
    if NREC == 1:
        gcoef, mixc, mix_lo = 1.0, None, None
    elif alpha <= 0.5:
        gcoef, mixc, mix_lo = 1.0 - alpha, alpha / (1.0 - alpha), True
    else:
        gcoef, mixc, mix_lo = alpha, (1.0 - alpha) / alpha, False

    nc = bacc.Bacc("TRN2", target_bir_lowering=False, debug=False,
                   num_devices=N_CORES)

    xT_d = nc.declare_dram_parameter("xT", [2, 128, TOK], dt.float32r, isOutput=False)
    pw_d = nc.declare_dram_parameter("pw", [128, NMAT * 2 * 4 * 128], dt.float32r, isOutput=False)
    bias_d = nc.declare_dram_parameter("bias", [128, 4 * NMAT + 4], dt.float32, isOutput=False)
    aw_d = nc.declare_dram_parameter("aw", [128, NREC * 16 * 128], dt.float16, isOutput=False)
    s0_d = nc.declare_dram_parameter("s0T", [128, 64], dt.float16, isOutput=False)
    iden_d = nc.declare_dram_parameter("iden", [128, 128], dt.float16, isOutput=False)
    stg_d = nc.declare_dram_parameter("stg", [128, T * 64], dt.float16, isOutput=True)

    with TileContext(nc) as tc:
      with tc.tile_pool(name="dram", bufs=1, space="DRAM") as dpool:
        # bx staging: col = t*(NREC*64) + m*64 + j*16 + b
        pjb_d = dpool.tile([128, T * NREC * 64], dt.float16, tag="pjbd",
                           name="pjbd")
        # gate staging: col = t*128 + kind*64 + j*16 + b  (0=gco, 1=g1m)
        gat_d = dpool.tile([128, T * 128], dt.float16, tag="gatd", name="gatd")

        with (
            tc.tile_pool(name="wp", bufs=1) as wp,
            tc.tile_pool(name="p1x", bufs=3) as p1x,
            tc.tile_pool(name="p1o", bufs=3) as p1o,
            tc.tile_pool(name="p1ps", bufs=3, space="PSUM") as p1ps,
            tc.tile_pool(name="p2in", bufs=2) as p2in,
            tc.tile_pool(name="p2stg", bufs=2) as p2stg,
            tc.tile_pool(name="p2c", bufs=16) as p2c,
            tc.tile_pool(name="p2ps", bufs=2, space="PSUM") as p2ps,
        ):
            pw = wp.tile([128, NMAT * 2 * 4 * 128], dt.float32r)
            nc.sync.dma_start(pw[:], pw_d[:])
            bias = wp.tile([128, 4 * NMAT + 4], dt.float32)
            nc.sync.dma_start(bias[:], bias_d[:])
            gcoef_t = wp.tile([128, 256], dt.float16)
            nc.gpsimd.memset(gcoef_t[:], gcoef)
            aw = wp.tile([128, NREC * 16 * 128], dt.float16)
            nc.sync.dma_start(aw[:], aw_d[:])
            iden = wp.tile([128, 128], dt.float16)
            nc.sync.dma_start(iden[:], iden_d[:])
            s0sb = wp.tile([128, 64], dt.float16)
            nc.sync.dma_start(s0sb[:], s0_d[:])

            # ---------------- projection emitters ----------------
            p1st = {}

            def p1_prefetch(tt):
                if tt >= NTT:
                    return
                xt = p1x.tile([128, 2 * 512], dt.float32r, tag="xt")
                for i in range(2):
                    nc.sync.dma_start(xt[:, i * 512:(i + 1) * 512],
                                      xT_d[i, :, tt * 512:(tt + 1) * 512])
                p1st[tt] = {"xt": xt}

            def p1_unit(tt, u):
                """u in 0..NMAT*4-1 -> (mat, j): 2 matmuls + post ops."""
                if tt >= NTT:
                    return
                st = p1st[tt]
                if u == 0:
                    st["pjbpk"] = p1o.tile([128, 32 * NREC * 64], dt.float16,
                                           tag="pjbpk", name="pjbpk")
                    st["gatpk"] = p1o.tile([128, 32 * 128], dt.float16,
                                           tag="gatpk", name="gatpk")
                mat, j = divmod(u, 4)
                pjbpk_r = st["pjbpk"][:].rearrange(
                    "p (t m j b) -> p t m j b", t=32, m=NREC, j=4)
                gatpk_r = st["gatpk"][:].rearrange(
                    "p (t k j b) -> p t k j b", t=32, k=2, j=4)
                ps = p1ps.tile([128, 512], dt.float32, tag="pps")
                for i in range(2):
                    blk = ((mat * 2 + i) * 4 + j) * 128
                    nc.tensor.matmul(ps[:], pw[:, blk:blk + 128],
                                     st["xt"][:, i * 512:(i + 1) * 512],
                                     start=(i == 0), stop=(i == 1))
                psr = ps[:].rearrange("p (t b) -> p t b", t=32)
                bj = bias[:, mat * 4 + j:mat * 4 + j + 1]
                if mat == NMAT - 1:
                    g16 = p1o.tile([128, 512], dt.float16, tag="g16")
                    g16r = g16[:].rearrange("p (t b) -> p t b", t=32)
                    nbj = bias[:, 4 * NMAT + j:4 * NMAT + j + 1]
                    for h in range(2):
                        hs = slice(h * 16, (h + 1) * 16)
                        nc.scalar.activation(g16r[:, hs], psr[:, hs],
                                             AF.Sigmoid, bias=bj, scale=1.0)
                        nc.scalar.activation(gatpk_r[:, hs, 1, j, :],
                                             psr[:, hs], AF.Sigmoid,
                                             bias=nbj, scale=-1.0)
                        nc.gpsimd.tensor_tensor(
                            gatpk_r[:, hs, 0, j, :], g16r[:, hs],
                            gcoef_t[:].rearrange("p (t b) -> p t b", t=16),
                            ALU.mult)
                else:
                    for h in range(2):
                        hs = slice(h * 16, (h + 1) * 16)
                        nc.vector.tensor_scalar(pjbpk_r[:, hs, mat, j, :],
                                                psr[:, hs], bj, None, ALU.add)

            def p1_flush(tt):
                if tt >= NTT:
                    return
                st = p1st.pop(tt)
                GW32 = 32 * NREC * 64
                nc.sync.dma_start(pjb_d[:, tt * GW32:(tt + 1) * GW32],
                                  st["pjbpk"][:])
                nc.sync.dma_start(gat_d[:, tt * 4096:(tt + 1) * 4096],
                                  st["gatpk"][:])

            # prologue: first LAG tiles of projections
            NU = NMAT * 4
            for tt in range(min(LAG, NTT)):
                p1_prefetch(tt)
                for u in range(NU):
                    p1_unit(tt, u)
                p1_flush(tt)
            p1_prefetch(LAG)

            # ---------------- recurrence ----------------
            GW = 16 * NREC * 64
            prev_stg = None
            for g in range(NG):
                pjb = p2in.tile([128, GW], dt.float16, tag="pjb")
                nc.sync.dma_start(pjb[:], pjb_d[:, g * GW:(g + 1) * GW])
                pjb_r = pjb[:].rearrange("p (t m j b) -> p m j t b",
                                         t=16, m=NREC, j=4)
                gat = p2in.tile([128, 2048], dt.float16, tag="gat")
                nc.sync.dma_start(gat[:], gat_d[:, g * 2048:(g + 1) * 2048])

                stg = p2stg.tile([128, 16 * 64], dt.float16, tag="stg")

                # projection work interleaved into this group
                tile_idx = g // 2 + LAG
                ubase = 0 if g % 2 == 0 else NU - NU // 2
                units = list(range(ubase, min(ubase + NU - NU // 2, NU)))

                for tt in range(16):
                    if g == 0 and tt == 0:
                        st_tile, st_off = s0sb, 0
                    elif tt == 0:
                        st_tile, st_off = prev_stg, 15 * 64
                    else:
                        st_tile, st_off = stg, (tt - 1) * 64

                    # m2 halves on GPSIMD (off the critical DVE chain)
                    m2a = p2c.tile([128, 32], dt.float16, tag="m2a")
                    nc.gpsimd.tensor_tensor(
                        m2a[:], st_tile[:, st_off:st_off + 32],
                        gat[:, tt * 128 + 64:tt * 128 + 96], ALU.mult)
                    m2b = p2c.tile([128, 32], dt.float16, tag="m2b")
                    nc.gpsimd.tensor_tensor(
                        m2b[:], st_tile[:, st_off + 32:st_off + 64],
                        gat[:, tt * 128 + 96:tt * 128 + 128], ALU.mult)

                    # two psum pairs, cols (m, jj, b); idens + early matmuls
                    # (k=0,1) first, then the late batch (k=2,3)
                    psps = []
                    for p in range(2):
                        psp = p2ps.tile([128, 2 * W], dt.float32,
                                        tag=f"psp{p}", name=f"psp{p}")
                        psps.append(psp)
                        nc.tensor.matmul(
                            psp[:].rearrange("q (m j b) -> q m j b",
                                             m=NREC, j=2),
                            iden[:], pjb_r[:, :, 2 * p:2 * p + 2, tt, :],
                            start=True, stop=False)
                        for jj in range(2):
                            j = 2 * p + jj
                            for m in range(NREC):
                                for k in (0, 1):
                                    blk = (m * 16 + k * 4 + j) * 128
                                    nc.tensor.matmul(
                                        psp[:, (m * 2 + jj) * 16:
                                            (m * 2 + jj + 1) * 16],
                                        aw[:, blk:blk + 128],
                                        st_tile[:, st_off + k * 16:
                                                st_off + (k + 1) * 16],
                                        start=False, stop=False)
                    fts = []
                    for p in range(2):
                        psp = psps[p]
                        for jj in range(2):
                            j = 2 * p + jj
                            for m in range(NREC):
                                for k in (2, 3):
                                    blk = (m * 16 + k * 4 + j) * 128
                                    nc.tensor.matmul(
                                        psp[:, (m * 2 + jj) * 16:
                                            (m * 2 + jj + 1) * 16],
                                        aw[:, blk:blk + 128],
                                        st_tile[:, st_off + k * 16:
                                                st_off + (k + 1) * 16],
                                        start=False, stop=(k == 3))
                        ft = p2c.tile([128, 2 * W], dt.float16,
                                      tag=f"ftp{p}", name=f"ftp{p}")
                        nc.scalar.activation(ft[:], psp[:], AF.Tanh)
                        fts.append(ft)

                    # tails: pair0's chain has priority on DVE (its state
                    # feeds the next step's early matmuls); pair1's blend
                    # finishes on GPSIMD
                    us = []
                    for p in range(2):
                        ft = fts[p]
                        if NREC == 2:
                            u = p2c.tile([128, 32], dt.float16,
                                         tag=f"up{p}", name=f"up{p}")
                            if mix_lo:
                                nc.vector.scalar_tensor_tensor(
                                    u[:], ft[:, 32:64], mixc, ft[:, 0:32],
                                    ALU.mult, ALU.add)
                            else:
                                nc.vector.scalar_tensor_tensor(
                                    u[:], ft[:, 0:32], mixc, ft[:, 32:64],
                                    ALU.mult, ALU.add)
                        else:
                            u = ft
                        us.append(u)
                        eng = nc.vector if p == 0 else nc.gpsimd
                        m2h = m2a if p == 0 else m2b
                        mc = p2c.tile([128, 32], dt.float16,
                                      tag=f"mcp{p}", name=f"mcp{p}")
                        eng.tensor_tensor(
                            mc[:], u[:, 0:32],
                            gat[:, tt * 128 + p * 32:tt * 128 + p * 32 + 32],
                            ALU.mult)
                        eng.tensor_tensor(
                            stg[:, tt * 64 + p * 32:tt * 64 + p * 32 + 32],
                            mc[:], m2h[:], ALU.add)

                    # projection work in the PE stall window at step end
                    if tt % 2 == 1 and tt // 2 < len(units):
                        p1_unit(tile_idx, units[tt // 2])
                    if tt == 14 and g % 2 == 1:
                        p1_flush(tile_idx)
                        p1_prefetch(tile_idx + 1)

                nc.sync.dma_start(stg_d[:, g * 1024:(g + 1) * 1024], stg[:])
                prev_stg = stg

    nc.compile()
    return nc


def _pack_lhsT_blocks(W, kdim, mdim, dtype):
    """W: [mdim*128, kdim*128]; returns [128, kdim*mdim*128] with block
    (k, j) at cols (k*mdim+j)*128 equal to W[j-chunk, k-chunk].T."""
    nk, nj = kdim, mdim
    out = np.zeros((128, nk * nj * 128), dtype=dtype)
    for k in range(nk):
        for j in range(nj):
            blk = W[j * 128:(j + 1) * 128, k * 128:(k + 1) * 128].T
            out[:, (k * nj + j) * 128:(k * nj + j + 1) * 128] = blk
    return np.ascontiguousarray(out)


def kernel(x_seq, s0, A0_w, B0_w, B0_b, A1_w, B1_w, B1_b, gate_w, gate_b,
           alpha, z, _T=None, _trace=False):
    from concourse.bass_utils import run_bass_kernel_spmd

    T = int(_T or T_FULL)
    alpha_f = float(np.asarray(alpha))
    z_i = int(np.asarray(z))

    key = (alpha_f, z_i, T)
    if key not in _cache:
        _cache[key] = _build(alpha_f, z_i, T)
    nc = _cache[key]

    NREC = 2 if z_i != 0 else 1
    NMAT = NREC + 1

    x_seq = np.asarray(x_seq, dtype=np.float32)
    s0 = np.asarray(s0, dtype=np.float32)

    # ---- shared (replicated) weight packing ----
    mats = [np.asarray(B0_w), np.asarray(B1_w), np.asarray(gate_w)] \
        if z_i != 0 else [np.asarray(B0_w), np.asarray(gate_w)]
    biases = [np.asarray(B0_b), np.asarray(B1_b), np.asarray(gate_b)] \
        if z_i != 0 else [np.asarray(B0_b), np.asarray(gate_b)]
    pw = np.concatenate(
        [_pack_lhsT_blocks(W.astype(np.float32), 2, 4, np.float32)
         for W in mats], axis=1)
    pw = np.ascontiguousarray(pw)

    bias = np.zeros((128, 4 * NMAT + 4), np.float32)
    for mi, bvec in enumerate(biases):
        bias[:, mi * 4:(mi + 1) * 4] = bvec.astype(np.float32).reshape(4, 128).T
    # negated gate bias (for g1m = sigmoid(-x) on ACT with scale=-1)
    bias[:, 4 * NMAT:] = -bias[:, (NMAT - 1) * 4:NMAT * 4]

    recs = [np.asarray(A0_w)] if z_i == 0 else [np.asarray(A0_w), np.asarray(A1_w)]
    aw = np.concatenate(
        [_pack_lhsT_blocks(A.astype(np.float32), 4, 4, np.float32)
         for A in recs], axis=1).astype(np.float16)
    aw = np.ascontiguousarray(aw)

    IDEN = np.ascontiguousarray(np.eye(128, dtype=np.float16))

    # ---- per-core inputs ----
    in_maps = []
    for c in range(N_CORES):
        bc = c * B_LOC
        xc = x_seq[bc:bc + B_LOC, :T]                       # [16, T, 256]
        xT = np.ascontiguousarray(
            xc.transpose(2, 1, 0).reshape(2, 128, T * B_LOC))
        s0c = s0[bc:bc + B_LOC]                             # [16, 512]
        s0T = np.ascontiguousarray(
            s0c.T.reshape(4, 128, B_LOC).transpose(1, 0, 2).reshape(128, 64)
        ).astype(np.float16)
        in_maps.append({
            "xT": xT, "pw": pw, "bias": bias, "aw": aw, "s0T": s0T,
            "iden": IDEN,
        })

    res = run_bass_kernel_spmd(nc, in_maps, list(range(N_CORES)), trace=_trace)
    if _trace:
        kernel._last_res = res

    out = np.empty((B_FULL, T + 1, S_DIM), np.float32)
    for c in range(N_CORES):
        bc = c * B_LOC
        stg = res.results[c]["stg"]                         # [128, T*64] fp16
        out[bc:bc + B_LOC, 0] = s0[bc:bc + B_LOC]
        out[bc:bc + B_LOC, 1:] = (
            stg.reshape(128, T, 4, B_LOC).transpose(3, 1, 2, 0)
            .reshape(B_LOC, T, S_DIM).astype(np.float32))
    return out
